# revision 10
# baseline (speedup 1.0000x reference)
"""PoseVelGraph residuals on 8 Trainium2 NeuronCores.

Strategy (see sharding_hint): shard edges/poses data-parallel across the 8
cores; each edge shard carries its endpoint node features (edge-cut GNN
distribution, host attaches nodes[edges[:,0]]/nodes[edges[:,1]] during
sharding).  The [M,*] IMU chain shards along the node axis; the 1-element
halo for diff() is handled by shipping row r and row r+1 slabs.

On-device layout: SoA "component blocks" per partition — a tile [128, C*L]
holds C component blocks of L contiguous elements, so the vector algebra runs
as wide [128, c*L] slab ops.  Precision split:
  - quaternion chain (q inputs, quaternion products, so3-log scalar chain) in
    fp32: the relative-rotation vector part comes from cancelling O(1) terms,
    and near-identity relative rotations (tiny |v|, huge V^-1 coefficient)
    amplify absolute errors, so fp16 there fails;
  - translation chain (t2-t1, rotations of translations, tau assembly) and
    the adjacent-velocity / trans-velocity residuals in fp16 (2x DVE packing).

Residuals per edge e and chain row r:
  pgerr     = se3_log( poses^-1 o n1^-1 o n2 )              [E,6]
  adjvelerr = 0.1 * (imu_dvels - (vels[1:] - vels[:-1]))    [M,3]
  imuroterr = so3_log( drot^-1 o q[:-1]^-1 o q[1:] )        [M,3]
  transvelerr = 0.1 * ((t[1:]-t[:-1]) - (vels[:-1]*dts + imu_dtrans))
Output = concat of the four raveled blocks.
"""

import numpy as np

import concourse.bass as bass
import concourse.mybir as mybir
from concourse.tile import TileContext

F16 = mybir.dt.float16
F32 = mybir.dt.float32
OP = mybir.AluOpType
AF = mybir.ActivationFunctionType
P = 128
PI = float(np.pi)
EPS = 1e-8

LAST_RESULT = None  # BassKernelResults of the most recent run (for harness)

# full-problem config
FULL = dict(E=2_000_000, N=1_000_000, M=999_999, K=490, NT=4, NS=2, NC=8)


def _split_excess_waits(nc, max_waits=1):
    """walrus CoreV3 codegen rejects instructions carrying several sem waits
    ("Too many sync wait commands").  Hoist excess waits onto same-engine
    NoOps placed just before the instruction; per-engine in-order execution
    makes this equivalent."""
    for f in nc.m.functions:
        for bb in f.blocks:
            new = []
            for ins in bb.instructions:
                si = ins.sync_info
                w = list(si.on_wait) if si and si.on_wait else []
                if len(w) > max_waits:
                    keep = w[-max_waits:]
                    extras = w[:-max_waits]
                    for i in range(0, len(extras), max_waits):
                        chunk = extras[i:i + max_waits]
                        nop = mybir.InstNoOp(
                            name=f"I-waitfix-{nc.next_id()}",
                            engine=ins.engine,
                            sync_info=mybir.SyncInfo(on_wait=chunk, on_update=[]),
                        )
                        new.append(nop)
                    si.on_wait = keep
                new.append(ins)
            bb.instructions[:] = new


class _Emit:
    def __init__(self, nc, pools, L):
        self.nc = nc
        self.pools = pools
        self.L = L
        self.V = nc.vector
        self.A = nc.scalar
        self.S = nc.sync
        self._uid = 0

    # ---- rotating-tag tile allocation ----
    def _t(self, pool, shape, dt, tag):
        self._uid += 1
        return self.pools[pool].tile(shape, dt, name=f"{tag}_{self._uid}", tag=tag)

    def d6(self):       # fp16 dup6 (cross-product rotated views)
        return self._t("d6", [P, 6 * self.L], F16, "d6")

    def s3(self):       # fp16 3-block slab
        return self._t("s3", [P, 3 * self.L], F16, "s3")

    def s3f(self):      # fp32 3-block slab
        return self._t("s3f", [P, 3 * self.L], F32, "s3f")

    def s1(self):
        return self._t("s1", [P, self.L], F16, "s1")

    def sk(self):
        return self._t("sk", [P, self.L], F32, "sk")

    def ski(self):      # int32 (CopyPredicated wants an integer mask)
        return self._t("sk", [P, self.L], mybir.dt.int32, "sk")

    def blk(self, ap, i, n=1):
        return ap[:, i * self.L:(i + n) * self.L]

    def dup(self, d6t):
        self.A.copy(self.blk(d6t, 3, 3), self.blk(d6t, 0, 3))

    def bcast3(self, out3, src1):
        for c in range(3):
            self.A.copy(self.blk(out3, c), src1)

    # ---- fp32 quaternion product, component-wise (no dup tiles needed) ----
    def qmul_conj_c(self, vA3, wAk, vB3, wBk, out_v3, out_wk):
        """out = conj(A) (x) B, fp32.
        vA3/vB3: contiguous [128,3L] f32 APs (x,y,z blocks); wAk/wBk [128,L].
        out_w = wA*wB + dot(vA,vB);  out_v = wA*vB - wB*vA - vA x vB."""
        V, b = self.V, self.blk
        pv = self.s3f()
        V.tensor_tensor(out=pv[:], in0=vA3, in1=vB3, op=OP.mult)
        s1a = self.sk()
        V.tensor_tensor(out=s1a[:], in0=b(pv, 0), in1=b(pv, 1), op=OP.add)
        pw = self.sk()
        V.tensor_tensor(out=pw[:], in0=wAk, in1=wBk, op=OP.mult)
        V.tensor_tensor(out=s1a[:], in0=s1a[:], in1=b(pv, 2), op=OP.add)
        V.tensor_tensor(out=out_wk, in0=s1a[:], in1=pw[:], op=OP.add)

        def vc(v3, c):
            return v3[:, (c % 3) * self.L:((c % 3) + 1) * self.L]

        for c in range(3):
            oc = vc(out_v3, c)
            V.tensor_tensor(out=oc, in0=wAk, in1=vc(vB3, c), op=OP.mult)
            t1 = self.sk()
            V.tensor_tensor(out=t1[:], in0=wBk, in1=vc(vA3, c), op=OP.mult)
            V.tensor_tensor(out=oc, in0=oc, in1=t1[:], op=OP.subtract)
            V.tensor_tensor(out=t1[:], in0=vc(vA3, c + 1), in1=vc(vB3, c + 2),
                            op=OP.mult)
            V.tensor_tensor(out=oc, in0=oc, in1=t1[:], op=OP.subtract)
            V.tensor_tensor(out=t1[:], in0=vc(vA3, c + 2), in1=vc(vB3, c + 1),
                            op=OP.mult)
            V.tensor_tensor(out=oc, in0=oc, in1=t1[:], op=OP.add)

    # ---- fp16 slab cross / rotation (translation chain) ----
    def cross(self, a6, b6, out3):
        V, b = self.V, self.blk
        t2 = self.s3()
        V.tensor_tensor(out=out3, in0=b(a6, 1, 3), in1=b(b6, 2, 3), op=OP.mult)
        V.tensor_tensor(out=t2[:], in0=b(a6, 2, 3), in1=b(b6, 1, 3), op=OP.mult)
        V.tensor_tensor(out=out3, in0=out3, in1=t2[:], op=OP.subtract)

    def qrot_conj(self, v6, w3, u6, out3):
        """out = R(conj(q))*u = u - 2w*(v x u) + 2*(v x (v x u)), fp16."""
        V, b = self.V, self.blk
        c1d6 = self.d6()
        self.cross(v6, u6, b(c1d6, 0, 3))
        self.dup(c1d6)
        c2 = self.s3()
        self.cross(v6, c1d6, c2[:])
        mw = self.s3()
        V.tensor_tensor(out=mw[:], in0=w3[:], in1=b(c1d6, 0, 3), op=OP.mult)
        V.scalar_tensor_tensor(out=out3, in0=mw[:], scalar=-2.0, in1=b(u6, 0, 3),
                               op0=OP.mult, op1=OP.add)
        V.scalar_tensor_tensor(out=out3, in0=c2[:], scalar=2.0, in1=out3,
                               op0=OP.mult, op1=OP.add)

    def so3_se3(self, qv3, qwk, out_phi3, te6=None, out_tau3=None,
                phi_s3=None):
        """phi = so3_log(q) written to fp16 out_phi3; optional se3 tau.
        qv3/qwk are fp32.  atan2 via the swap trick keeps |atan arg| <= 1
        (ACT table valid range).  For unit q: cos=2w^2-1, sin=2nw, so the
        V^-1 coefficient reduces to 1/th^2 - w/(2 th n)."""
        V, A, b, L = self.V, self.A, self.blk, self.L
        sqx, sqy, sqz = self.sk(), self.sk(), self.sk()
        A.activation(out=sqx[:], in_=qv3[:, 0:L], func=AF.Square)
        A.activation(out=sqy[:], in_=qv3[:, L:2 * L], func=AF.Square)
        A.activation(out=sqz[:], in_=qv3[:, 2 * L:3 * L], func=AF.Square)
        n2 = self.sk()
        V.tensor_tensor(out=n2[:], in0=sqx[:], in1=sqy[:], op=OP.add)
        V.tensor_tensor(out=n2[:], in0=n2[:], in1=sqz[:], op=OP.add)
        n_ = self.sk()
        A.activation(out=n_[:], in_=n2[:], func=AF.Sqrt)
        nG = self.sk()
        V.tensor_scalar(out=nG[:], in0=n_[:], scalar1=1e-20, scalar2=None,
                        op0=OP.max)
        rn = self.sk()
        V.reciprocal(out=rn[:], in_=nG[:])
        rw = self.sk()
        V.reciprocal(out=rw[:], in_=qwk)
        aw = self.sk()
        A.activation(out=aw[:], in_=qwk, func=AF.Abs)
        m1 = self.ski()
        V.tensor_tensor(out=m1[:], in0=n_[:], in1=aw[:], op=OP.is_ge)
        a1 = self.sk()
        V.tensor_tensor(out=a1[:], in0=qwk, in1=rn[:], op=OP.mult)
        a2 = self.sk()
        V.tensor_tensor(out=a2[:], in0=n_[:], in1=rw[:], op=OP.mult)
        arg = self.sk()
        V.select(out=arg[:], mask=m1[:], on_true=a1[:], on_false=a2[:])
        atA = self.sk()
        A.activation(out=atA[:], in_=arg[:], func=AF.Arctan)
        thA = self.sk()
        V.tensor_scalar(out=thA[:], in0=atA[:], scalar1=-2.0, scalar2=PI,
                        op0=OP.mult, op1=OP.add)
        ngm = self.sk()
        V.tensor_scalar(out=ngm[:], in0=qwk, scalar1=0.0, scalar2=None,
                        op0=OP.is_lt)
        at2 = self.sk()
        V.tensor_scalar(out=at2[:], in0=atA[:], scalar1=2.0, scalar2=None,
                        op0=OP.mult)
        thB = self.sk()
        V.scalar_tensor_tensor(out=thB[:], in0=ngm[:], scalar=2.0 * PI,
                               in1=at2[:], op0=OP.mult, op1=OP.add)
        th = self.sk()
        V.select(out=th[:], mask=m1[:], on_true=thA[:], on_false=thB[:])
        kb = self.sk()
        V.tensor_tensor(out=kb[:], in0=th[:], in1=rn[:], op=OP.mult)
        ks = self.sk()
        V.tensor_scalar(out=ks[:], in0=rw[:], scalar1=2.0, scalar2=None,
                        op0=OP.mult)
        mn = self.ski()
        V.tensor_scalar(out=mn[:], in0=n_[:], scalar1=EPS, scalar2=None,
                        op0=OP.is_gt)
        kk = self.sk()
        V.select(out=kk[:], mask=mn[:], on_true=kb[:], on_false=ks[:])
        kk3 = self.s3f()
        self.bcast3(kk3, kk[:])
        # phi (fp32 product, fp16 store)
        if phi_s3 is not None:
            V.tensor_tensor(out=phi_s3, in0=qv3, in1=kk3[:], op=OP.mult)
            A.copy(out_phi3, phi_s3)
        else:
            V.tensor_tensor(out=out_phi3, in0=qv3, in1=kk3[:], op=OP.mult)
        if te6 is None:
            return
        thG = self.sk()
        V.tensor_scalar(out=thG[:], in0=th[:], scalar1=1e-6, scalar2=None,
                        op0=OP.max)
        k1 = self.sk()
        V.reciprocal(out=k1[:], in_=thG[:])
        dd = self.sk()
        V.scalar_tensor_tensor(out=dd[:], in0=a1[:], scalar=-0.5, in1=k1[:],
                               op0=OP.mult, op1=OP.add)
        coefb = self.sk()
        V.tensor_tensor(out=coefb[:], in0=dd[:], in1=k1[:], op=OP.mult)
        ms = self.sk()
        V.tensor_scalar(out=ms[:], in0=th[:], scalar1=1e-4, scalar2=None,
                        op0=OP.is_lt)
        u1 = self.sk()
        V.scalar_tensor_tensor(out=u1[:], in0=coefb[:], scalar=-1.0, in1=ms[:],
                               op0=OP.mult, op1=OP.mult)
        u2 = self.sk()
        V.scalar_tensor_tensor(out=u2[:], in0=ms[:], scalar=1.0 / 12.0,
                               in1=coefb[:], op0=OP.mult, op1=OP.add)
        coef = self.sk()
        V.tensor_tensor(out=coef[:], in0=u1[:], in1=u2[:], op=OP.add)
        coef3 = self.s3()
        self.bcast3(coef3, coef[:])
        phi6 = self.d6()
        A.copy(b(phi6, 0, 3), out_phi3)
        self.dup(phi6)
        px6 = self.d6()
        self.cross(phi6, te6, b(px6, 0, 3))
        self.dup(px6)
        cpp = self.s3()
        self.cross(phi6, px6, cpp[:])
        gg = self.s3()
        V.scalar_tensor_tensor(out=gg[:], in0=b(px6, 0, 3), scalar=-0.5,
                               in1=b(te6, 0, 3), op0=OP.mult, op1=OP.add)
        hh = self.s3()
        V.tensor_tensor(out=hh[:], in0=coef3[:], in1=cpp[:], op=OP.mult)
        V.tensor_tensor(out=out_tau3, in0=gg[:], in1=hh[:], op=OP.add)


def build_nc(cfg):
    K, NT, NS = cfg["K"], cfg["NT"], cfg["NS"]
    nc = bass.Bass()
    din = {}
    for nm, sh, dt in [
        ("enq1", [NT, P, 4 * K], F32), ("enq2", [NT, P, 4 * K], F32),
        ("epq", [NT, P, 4 * K], F32),
        ("ent1", [NT, P, 3 * K], F16), ("ent2", [NT, P, 3 * K], F16),
        ("ept", [NT, P, 3 * K], F16),
        ("inq0", [NS, P, 4 * K], F32), ("inq1", [NS, P, 4 * K], F32),
        ("idrq", [NS, P, 4 * K], F32),
        ("int0", [NS, P, 3 * K], F16), ("int1", [NS, P, 3 * K], F16),
        ("iv0", [NS, P, 3 * K], F16), ("iv1", [NS, P, 3 * K], F16),
        ("idtr", [NS, P, 3 * K], F16), ("idv", [NS, P, 3 * K], F16),
        ("idts", [NS, P, K], F16),
    ]:
        din[nm] = nc.dram_tensor(nm, sh, dt, kind="ExternalInput")
    pg = nc.dram_tensor("pg", [NT, P, 6 * K], F16, kind="ExternalOutput")
    adj = nc.dram_tensor("adj", [NS, P, 3 * K], F16, kind="ExternalOutput")
    rot = nc.dram_tensor("rot", [NS, P, 3 * K], F16, kind="ExternalOutput")
    tvl = nc.dram_tensor("tvl", [NS, P, 3 * K], F16, kind="ExternalOutput")

    with TileContext(nc) as tc:
        with (
            tc.tile_pool(name="io_e", bufs=1) as io_e,
            tc.tile_pool(name="io_o", bufs=2) as io_o,
            tc.tile_pool(name="io_i", bufs=1) as io_i,
            tc.tile_pool(name="d6", bufs=5) as d6p,
            tc.tile_pool(name="s3", bufs=8) as s3p,
            tc.tile_pool(name="s3f", bufs=5) as s3fp,
            tc.tile_pool(name="s1", bufs=4) as s1p,
            tc.tile_pool(name="sk", bufs=12) as skp,
        ):
            pools = dict(d6=d6p, s3=s3p, s3f=s3fp, s1=s1p, sk=skp)
            em = _Emit(nc, pools, K)
            V, A, S, b = em.V, em.A, em.S, em.blk

            def edge_tile(t):
                q1 = io_e.tile([P, 4 * K], F32, name=f"q1_{t}", tag="q1")
                q2 = io_e.tile([P, 4 * K], F32, name=f"q2_{t}", tag="q2")
                qp = io_e.tile([P, 4 * K], F32, name=f"qp_{t}", tag="qp")
                t1 = io_e.tile([P, 3 * K], F16, name=f"t1_{t}", tag="t1")
                t2 = io_e.tile([P, 3 * K], F16, name=f"t2_{t}", tag="t2")
                tp = io_e.tile([P, 3 * K], F16, name=f"tp_{t}", tag="tp")
                ot = io_o.tile([P, 6 * K], F16, name=f"ot_{t}", tag="ot")
                for tile_, nm in [(q1, "enq1"), (q2, "enq2"), (qp, "epq"),
                                  (t1, "ent1"), (t2, "ent2"), (tp, "ept")]:
                    S.dma_start(out=tile_[:], in_=din[nm][t, :, :])
                q1v, q1w = q1[:, 0:3 * K], q1[:, 3 * K:4 * K]
                q2v, q2w = q2[:, 0:3 * K], q2[:, 3 * K:4 * K]
                qpv, qpw = qp[:, 0:3 * K], qp[:, 3 * K:4 * K]
                # qa = conj(q1) (x) q2 ; qe = conj(qp) (x) qa   (fp32)
                qav = em.s3f()
                qaw = em.sk()
                em.qmul_conj_c(q1v, q1w, q2v, q2w, qav[:], qaw[:])
                qev = em.s3f()
                qew = em.sk()
                em.qmul_conj_c(qpv, qpw, qav[:], qaw[:], qev[:], qew[:])
                # fp16 copies of q1,qp for the translation rotations
                v16 = em.d6()
                A.copy(b(v16, 0, 3), q1v)
                em.dup(v16)
                w13 = em.s3()
                em.bcast3(w13, q1w)
                vp6 = em.d6()
                A.copy(b(vp6, 0, 3), qpv)
                em.dup(vp6)
                wp3 = em.s3()
                em.bcast3(wp3, qpw)
                # u = t2 - t1 ; ta = R(conj(q1)) u
                u6 = em.d6()
                V.tensor_tensor(out=b(u6, 0, 3), in0=t2[:], in1=t1[:],
                                op=OP.subtract)
                em.dup(u6)
                ta3 = em.s3()
                em.qrot_conj(v16, w13, u6, ta3[:])
                # v' = ta - tp ; te = R(conj(qp)) v'
                vv6 = em.d6()
                V.tensor_tensor(out=b(vv6, 0, 3), in0=ta3[:], in1=tp[:],
                                op=OP.subtract)
                em.dup(vv6)
                te6 = em.d6()
                em.qrot_conj(vp6, wp3, vv6, b(te6, 0, 3))
                em.dup(te6)
                em.so3_se3(qev[:], qew[:], b(ot[:], 3, 3), te6, b(ot[:], 0, 3))
                S.dma_start(out=pg[t, :, :], in_=ot[:])

            def imu_tile(s):
                tq0 = io_i.tile([P, 4 * K], F32, name=f"tq0_{s}", tag="tq0")
                tq1 = io_i.tile([P, 4 * K], F32, name=f"tq1_{s}", tag="tq1")
                tdr = io_i.tile([P, 4 * K], F32, name=f"tdr_{s}", tag="tdr")
                tt0 = io_i.tile([P, 3 * K], F16, name=f"tt0_{s}", tag="tt0")
                tt1 = io_i.tile([P, 3 * K], F16, name=f"tt1_{s}", tag="tt1")
                tv0 = io_i.tile([P, 3 * K], F16, name=f"tv0_{s}", tag="tv0")
                tv1 = io_i.tile([P, 3 * K], F16, name=f"tv1_{s}", tag="tv1")
                tdtr = io_i.tile([P, 3 * K], F16, name=f"tdtr_{s}", tag="tdtr")
                tdv = io_i.tile([P, 3 * K], F16, name=f"tdv_{s}", tag="tdv")
                tdts = io_i.tile([P, K], F16, name=f"tdts_{s}", tag="tdts")
                to_a = io_i.tile([P, 3 * K], F16, name=f"to_a_{s}", tag="to_a")
                to_r = io_i.tile([P, 3 * K], F16, name=f"to_r_{s}", tag="to_r")
                to_t = io_i.tile([P, 3 * K], F16, name=f"to_t_{s}", tag="to_t")
                for tile_, nm in [(tq0, "inq0"), (tq1, "inq1"), (tdr, "idrq"),
                                  (tt0, "int0"), (tt1, "int1"), (tv0, "iv0"),
                                  (tv1, "iv1"), (tdtr, "idtr"), (tdv, "idv"),
                                  (tdts, "idts")]:
                    S.dma_start(out=tile_[:], in_=din[nm][s, :, :])
                # part 2: adj = 0.1*(dv - (v1 - v0))
                dvv = em.s3()
                V.tensor_tensor(out=dvv[:], in0=tv1[:], in1=tv0[:], op=OP.subtract)
                ee = em.s3()
                V.tensor_tensor(out=ee[:], in0=tdv[:], in1=dvv[:], op=OP.subtract)
                A.mul(out=to_a[:], in_=ee[:], mul=0.1)
                S.dma_start(out=adj[s, :, :], in_=to_a[:])
                # part 4: tvl = 0.1*((t1 - t0) - (v0*dts + dtr))
                dts3 = em.s3()
                em.bcast3(dts3, tdts[:])
                y1 = em.s3()
                V.tensor_tensor(out=y1[:], in0=tv0[:], in1=dts3[:], op=OP.mult)
                V.tensor_tensor(out=y1[:], in0=y1[:], in1=tdtr[:], op=OP.add)
                y3 = em.s3()
                V.tensor_tensor(out=y3[:], in0=tt1[:], in1=tt0[:], op=OP.subtract)
                V.tensor_tensor(out=y3[:], in0=y3[:], in1=y1[:], op=OP.subtract)
                A.mul(out=to_t[:], in_=y3[:], mul=0.1)
                S.dma_start(out=tvl[s, :, :], in_=to_t[:])
                # part 3: qre = conj(dr) (x) (conj(q0) (x) q1), fp32
                qqv = em.s3f()
                qqw = em.sk()
                em.qmul_conj_c(tq0[:, 0:3 * K], tq0[:, 3 * K:4 * K],
                               tq1[:, 0:3 * K], tq1[:, 3 * K:4 * K],
                               qqv[:], qqw[:])
                qrv = em.s3f()
                qrw = em.sk()
                em.qmul_conj_c(tdr[:, 0:3 * K], tdr[:, 3 * K:4 * K],
                               qqv[:], qqw[:], qrv[:], qrw[:])
                em.so3_se3(qrv[:], qrw[:], to_r[:])
                S.dma_start(out=rot[s, :, :], in_=to_r[:])

            for t in range(NT):
                edge_tile(t)
            for s in range(NS):
                imu_tile(s)

    nc.finalize()
    _split_excess_waits(nc)
    return nc


# ---------------- host side ----------------

def _soa(x, ns, w, dt):
    """[ns*128*w, D] -> [ns, 128, D*w] component-block layout."""
    d = x.shape[1]
    return np.ascontiguousarray(
        x.reshape(ns, P, w, d).transpose(0, 1, 3, 2).reshape(ns, P, d * w)
    ).astype(dt)


def _unsoa(y, d, rows):
    ns, _, dw = y.shape
    w = dw // d
    out = y.astype(np.float32).reshape(ns, P, d, w).transpose(0, 1, 3, 2)
    return out.reshape(ns * P * w, d)[:rows]


def kernel(edges, nodes, vels, poses, imu_drots, imu_dtrans, imu_dvels, dts,
           cfg=None, _run=None):
    cfg = cfg or FULL
    E, N, M = cfg["E"], cfg["N"], cfg["M"]
    K, NT, NS, NC = cfg["K"], cfg["NT"], cfg["NS"], cfg["NC"]
    edges = np.asarray(edges)
    nodes = np.asarray(nodes, dtype=np.float32)
    vels = np.asarray(vels, dtype=np.float32)
    poses = np.asarray(poses, dtype=np.float32)
    imu_drots = np.asarray(imu_drots, dtype=np.float32)
    imu_dtrans = np.asarray(imu_dtrans, dtype=np.float32)
    imu_dvels = np.asarray(imu_dvels, dtype=np.float32)
    dts = np.asarray(dts, dtype=np.float32)

    Epc = E // NC
    Epad = P * K * NT
    MW = P * K * NS
    identq = np.array([0, 0, 0, 1], np.float32)

    # attach endpoint node features to each edge shard
    n1 = nodes[edges[:, 0]]
    n2 = nodes[edges[:, 1]]

    nodes_pad = np.vstack([
        nodes, np.tile(np.r_[np.zeros(3, np.float32), identq],
                       (NC * MW + 1 - N, 1))])
    vels_pad = np.vstack([vels, np.zeros((NC * MW + 1 - N, 3), np.float32)])
    mpad = NC * MW - M
    dr_pad = np.vstack([imu_drots, np.tile(identq, (mpad, 1))])
    dtr_pad = np.vstack([imu_dtrans, np.zeros((mpad, 3), np.float32)])
    dv_pad = np.vstack([imu_dvels, np.zeros((mpad, 3), np.float32)])
    dts_pad = np.vstack([dts, np.ones((mpad, 1), np.float32)])

    in_maps = []
    for c in range(NC):
        sl = slice(c * Epc, (c + 1) * Epc)

        def eq(x):  # [Epc,7] -> quaternion part tiles, f32
            xp = np.tile(identq, (Epad, 1))
            xp[:Epc] = x[sl][:, 3:7]
            return _soa(xp, NT, K, np.float32)

        def et(x):  # translation part tiles, f16
            xp = np.zeros((Epad, 3), np.float32)
            xp[:Epc] = x[sl][:, 0:3]
            return _soa(xp, NT, K, np.float16)

        base = c * MW
        m = dict(
            enq1=eq(n1), enq2=eq(n2), epq=eq(poses),
            ent1=et(n1), ent2=et(n2), ept=et(poses),
            inq0=_soa(nodes_pad[base:base + MW, 3:7], NS, K, np.float32),
            inq1=_soa(nodes_pad[base + 1:base + MW + 1, 3:7], NS, K, np.float32),
            idrq=_soa(dr_pad[base:base + MW], NS, K, np.float32),
            int0=_soa(nodes_pad[base:base + MW, 0:3], NS, K, np.float16),
            int1=_soa(nodes_pad[base + 1:base + MW + 1, 0:3], NS, K, np.float16),
            iv0=_soa(vels_pad[base:base + MW], NS, K, np.float16),
            iv1=_soa(vels_pad[base + 1:base + MW + 1], NS, K, np.float16),
            idtr=_soa(dtr_pad[base:base + MW], NS, K, np.float16),
            idv=_soa(dv_pad[base:base + MW], NS, K, np.float16),
            idts=_soa(dts_pad[base:base + MW], NS, K, np.float16),
        )
        in_maps.append(m)

    if _run is None:
        from concourse.bass_utils import run_bass_kernel_spmd
        nc = build_nc(cfg)
        res = run_bass_kernel_spmd(nc, in_maps, core_ids=list(range(NC)))
        global LAST_RESULT
        LAST_RESULT = res
        outs = res.results
    else:
        outs = _run(in_maps)

    pgs, adjs, rots, tvls = [], [], [], []
    for c in range(NC):
        o = outs[c]
        pgs.append(_unsoa(o["pg"].reshape(NT, P, 6 * K), 6, Epc))
        adjs.append(_unsoa(o["adj"], 3, MW))
        rots.append(_unsoa(o["rot"], 3, MW))
        tvls.append(_unsoa(o["tvl"], 3, MW))
    pg_full = np.concatenate(pgs, axis=0)
    adj_full = np.concatenate(adjs, axis=0)[:M]
    rot_full = np.concatenate(rots, axis=0)[:M]
    tvl_full = np.concatenate(tvls, axis=0)[:M]
    return np.concatenate([
        pg_full.ravel(), adj_full.ravel(), rot_full.ravel(), tvl_full.ravel()
    ]).astype(np.float32)


# revision 12
# speedup vs baseline: 40.7583x; 40.7583x over previous
"""PoseVelGraph residuals on 8 Trainium2 NeuronCores.

Strategy (see sharding_hint): shard edges/poses data-parallel across the 8
cores; each edge shard carries its endpoint node features (edge-cut GNN
distribution, host attaches nodes[edges[:,0]]/nodes[edges[:,1]] during
sharding).  The [M,*] IMU chain shards along the node axis; the 1-element
halo for diff() is handled by shipping row r and row r+1 slabs.

On-device layout: SoA "component blocks" per partition — a tile [128, C*L]
holds C component blocks of L contiguous elements, so the vector algebra runs
as wide [128, c*L] slab ops.  Precision split:
  - quaternion chain (q inputs, quaternion products, so3-log scalar chain) in
    fp32: the relative-rotation vector part comes from cancelling O(1) terms,
    and near-identity relative rotations (tiny |v|, huge V^-1 coefficient)
    amplify absolute errors, so fp16 there fails;
  - translation chain (t2-t1, rotations of translations, tau assembly) and
    the adjacent-velocity / trans-velocity residuals in fp16 (2x DVE packing).

Residuals per edge e and chain row r:
  pgerr     = se3_log( poses^-1 o n1^-1 o n2 )              [E,6]
  adjvelerr = 0.1 * (imu_dvels - (vels[1:] - vels[:-1]))    [M,3]
  imuroterr = so3_log( drot^-1 o q[:-1]^-1 o q[1:] )        [M,3]
  transvelerr = 0.1 * ((t[1:]-t[:-1]) - (vels[:-1]*dts + imu_dtrans))
Output = concat of the four raveled blocks.
"""

import numpy as np

import concourse.bass as bass
import concourse.mybir as mybir
from concourse.tile import TileContext

F16 = mybir.dt.float16
F32 = mybir.dt.float32
OP = mybir.AluOpType
AF = mybir.ActivationFunctionType
P = 128
PI = float(np.pi)
EPS = 1e-8

LAST_RESULT = None  # BassKernelResults of the most recent run (for harness)

# full-problem config
FULL = dict(E=2_000_000, N=1_000_000, M=999_999, K=490, NT=4, NS=2, NC=8)


def _split_excess_waits(nc, max_waits=1):
    """walrus CoreV3 codegen rejects instructions carrying several sem waits
    ("Too many sync wait commands").  Hoist excess waits onto same-engine
    NoOps placed just before the instruction; per-engine in-order execution
    makes this equivalent."""
    for f in nc.m.functions:
        for bb in f.blocks:
            new = []
            for ins in bb.instructions:
                si = ins.sync_info
                w = list(si.on_wait) if si and si.on_wait else []
                if len(w) > max_waits:
                    keep = w[-max_waits:]
                    extras = w[:-max_waits]
                    for i in range(0, len(extras), max_waits):
                        chunk = extras[i:i + max_waits]
                        nop = mybir.InstNoOp(
                            name=f"I-waitfix-{nc.next_id()}",
                            engine=ins.engine,
                            sync_info=mybir.SyncInfo(on_wait=chunk, on_update=[]),
                        )
                        new.append(nop)
                    si.on_wait = keep
                new.append(ins)
            bb.instructions[:] = new


class _Emit:
    def __init__(self, nc, pools, L, offload=None):
        self.nc = nc
        self.pools = pools
        self.L = L
        self.V = nc.vector
        self.A = nc.scalar
        self.S = nc.sync
        self.G = nc.gpsimd
        self.off = offload or {}
        self._uid = 0

    def eng(self, key):
        # engine for an offloadable fp16 op group: 'V' (default), 'G', 'A'
        return {"V": self.V, "G": self.G, "A": self.A}[self.off.get(key, "V")]

    # ---- rotating-tag tile allocation ----
    def _t(self, pool, shape, dt, tag):
        self._uid += 1
        return self.pools[pool].tile(shape, dt, name=f"{tag}_{self._uid}", tag=tag)

    def d6(self):       # fp16 dup6 (cross-product rotated views)
        return self._t("d6", [P, 6 * self.L], F16, "d6")

    def s3(self):       # fp16 3-block slab
        return self._t("s3", [P, 3 * self.L], F16, "s3")

    def s3f(self):      # fp32 3-block slab
        return self._t("s3f", [P, 3 * self.L], F32, "s3f")

    def s1(self):
        return self._t("s1", [P, self.L], F16, "s1")

    def sk(self):
        return self._t("sk", [P, self.L], F32, "sk")

    def ski(self):      # int32 (CopyPredicated wants an integer mask)
        return self._t("sk", [P, self.L], mybir.dt.int32, "sk")

    def blk(self, ap, i, n=1):
        return ap[:, i * self.L:(i + n) * self.L]

    def dup(self, d6t):
        self.A.copy(self.blk(d6t, 3, 3), self.blk(d6t, 0, 3))

    def bcast3(self, out3, src1):
        for c in range(3):
            self.A.copy(self.blk(out3, c), src1)

    # ---- fp32 quaternion product, component-wise (no dup tiles needed) ----
    def qmul_conj_c(self, vA3, wAk, vB3, wBk, out_v3, out_wk):
        """out = conj(A) (x) B, fp32.
        vA3/vB3: contiguous [128,3L] f32 APs (x,y,z blocks); wAk/wBk [128,L].
        out_w = wA*wB + dot(vA,vB);  out_v = wA*vB - wB*vA - vA x vB."""
        V, b = self.V, self.blk
        pv = self.s3f()
        V.tensor_tensor(out=pv[:], in0=vA3, in1=vB3, op=OP.mult)
        s1a = self.sk()
        V.tensor_tensor(out=s1a[:], in0=b(pv, 0), in1=b(pv, 1), op=OP.add)
        pw = self.sk()
        V.tensor_tensor(out=pw[:], in0=wAk, in1=wBk, op=OP.mult)
        V.tensor_tensor(out=s1a[:], in0=s1a[:], in1=b(pv, 2), op=OP.add)
        V.tensor_tensor(out=out_wk, in0=s1a[:], in1=pw[:], op=OP.add)

        def vc(v3, c):
            return v3[:, (c % 3) * self.L:((c % 3) + 1) * self.L]

        for c in range(3):
            oc = vc(out_v3, c)
            V.tensor_tensor(out=oc, in0=wAk, in1=vc(vB3, c), op=OP.mult)
            t1 = self.sk()
            V.tensor_tensor(out=t1[:], in0=wBk, in1=vc(vA3, c), op=OP.mult)
            V.tensor_tensor(out=oc, in0=oc, in1=t1[:], op=OP.subtract)
            V.tensor_tensor(out=t1[:], in0=vc(vA3, c + 1), in1=vc(vB3, c + 2),
                            op=OP.mult)
            V.tensor_tensor(out=oc, in0=oc, in1=t1[:], op=OP.subtract)
            V.tensor_tensor(out=t1[:], in0=vc(vA3, c + 2), in1=vc(vB3, c + 1),
                            op=OP.mult)
            V.tensor_tensor(out=oc, in0=oc, in1=t1[:], op=OP.add)

    # ---- fp16 slab cross / rotation (translation chain) ----
    def cross(self, a6, b6, out3, E=None):
        E, b = E or self.V, self.blk
        t2 = self.s3()
        E.tensor_tensor(out=out3, in0=b(a6, 1, 3), in1=b(b6, 2, 3), op=OP.mult)
        E.tensor_tensor(out=t2[:], in0=b(a6, 2, 3), in1=b(b6, 1, 3), op=OP.mult)
        E.tensor_tensor(out=out3, in0=out3, in1=t2[:], op=OP.subtract)

    def qrot_conj(self, v6, w3, u6, out3):
        """out = R(conj(q))*u = u - 2w*(v x u) + 2*(v x (v x u)), fp16."""
        V, b = self.V, self.blk
        E = self.eng("qrot")
        c1d6 = self.d6()
        self.cross(v6, u6, b(c1d6, 0, 3), E=E)
        self.dup(c1d6)
        c2 = self.s3()
        self.cross(v6, c1d6, c2[:], E=E)
        mw = self.s3()
        E.tensor_tensor(out=mw[:], in0=w3[:], in1=b(c1d6, 0, 3), op=OP.mult)
        V.scalar_tensor_tensor(out=out3, in0=mw[:], scalar=-2.0, in1=b(u6, 0, 3),
                               op0=OP.mult, op1=OP.add)
        V.scalar_tensor_tensor(out=out3, in0=c2[:], scalar=2.0, in1=out3,
                               op0=OP.mult, op1=OP.add)

    def so3_se3(self, qv3, qwk, out_phi3, te6=None, out_tau3=None,
                phi_s3=None):
        """phi = so3_log(q) written to fp16 out_phi3; optional se3 tau.
        qv3/qwk are fp32.  atan2 via the swap trick keeps |atan arg| <= 1
        (ACT table valid range).  For unit q: cos=2w^2-1, sin=2nw, so the
        V^-1 coefficient reduces to 1/th^2 - w/(2 th n)."""
        V, A, b, L = self.V, self.A, self.blk, self.L
        sqx, sqy, sqz = self.sk(), self.sk(), self.sk()
        A.activation(out=sqx[:], in_=qv3[:, 0:L], func=AF.Square)
        A.activation(out=sqy[:], in_=qv3[:, L:2 * L], func=AF.Square)
        A.activation(out=sqz[:], in_=qv3[:, 2 * L:3 * L], func=AF.Square)
        n2 = self.sk()
        V.tensor_tensor(out=n2[:], in0=sqx[:], in1=sqy[:], op=OP.add)
        V.tensor_tensor(out=n2[:], in0=n2[:], in1=sqz[:], op=OP.add)
        n_ = self.sk()
        A.activation(out=n_[:], in_=n2[:], func=AF.Sqrt)
        nG = self.sk()
        V.tensor_scalar(out=nG[:], in0=n_[:], scalar1=1e-20, scalar2=None,
                        op0=OP.max)
        rn = self.sk()
        V.reciprocal(out=rn[:], in_=nG[:])
        rw = self.sk()
        V.reciprocal(out=rw[:], in_=qwk)
        aw = self.sk()
        A.activation(out=aw[:], in_=qwk, func=AF.Abs)
        m1 = self.ski()
        V.tensor_tensor(out=m1[:], in0=n_[:], in1=aw[:], op=OP.is_ge)
        a1 = self.sk()
        V.tensor_tensor(out=a1[:], in0=qwk, in1=rn[:], op=OP.mult)
        a2 = self.sk()
        V.tensor_tensor(out=a2[:], in0=n_[:], in1=rw[:], op=OP.mult)
        arg = self.sk()
        A.copy(arg[:], a2[:])
        V.copy_predicated(out=arg[:], mask=m1[:], data=a1[:])
        atA = self.sk()
        A.activation(out=atA[:], in_=arg[:], func=AF.Arctan)
        thA = self.sk()
        V.tensor_scalar(out=thA[:], in0=atA[:], scalar1=-2.0, scalar2=PI,
                        op0=OP.mult, op1=OP.add)
        ngm = self.sk()
        V.tensor_scalar(out=ngm[:], in0=qwk, scalar1=0.0, scalar2=None,
                        op0=OP.is_lt)
        at2 = self.sk()
        V.tensor_scalar(out=at2[:], in0=atA[:], scalar1=2.0, scalar2=None,
                        op0=OP.mult)
        thB = self.sk()
        V.scalar_tensor_tensor(out=thB[:], in0=ngm[:], scalar=2.0 * PI,
                               in1=at2[:], op0=OP.mult, op1=OP.add)
        th = self.sk()
        A.copy(th[:], thB[:])
        V.copy_predicated(out=th[:], mask=m1[:], data=thA[:])
        kb = self.sk()
        V.tensor_tensor(out=kb[:], in0=th[:], in1=rn[:], op=OP.mult)
        ks = self.sk()
        V.tensor_scalar(out=ks[:], in0=rw[:], scalar1=2.0, scalar2=None,
                        op0=OP.mult)
        mn = self.ski()
        V.tensor_scalar(out=mn[:], in0=n_[:], scalar1=EPS, scalar2=None,
                        op0=OP.is_gt)
        kk = self.sk()
        A.copy(kk[:], ks[:])
        V.copy_predicated(out=kk[:], mask=mn[:], data=kb[:])
        kk3 = self.s3f()
        self.bcast3(kk3, kk[:])
        # phi (fp32 product, fp16 store)
        if phi_s3 is not None:
            V.tensor_tensor(out=phi_s3, in0=qv3, in1=kk3[:], op=OP.mult)
            A.copy(out_phi3, phi_s3)
        else:
            V.tensor_tensor(out=out_phi3, in0=qv3, in1=kk3[:], op=OP.mult)
        if te6 is None:
            return
        thG = self.sk()
        V.tensor_scalar(out=thG[:], in0=th[:], scalar1=1e-6, scalar2=None,
                        op0=OP.max)
        k1 = self.sk()
        V.reciprocal(out=k1[:], in_=thG[:])
        dd = self.sk()
        V.scalar_tensor_tensor(out=dd[:], in0=a1[:], scalar=-0.5, in1=k1[:],
                               op0=OP.mult, op1=OP.add)
        coefb = self.sk()
        V.tensor_tensor(out=coefb[:], in0=dd[:], in1=k1[:], op=OP.mult)
        ms = self.sk()
        V.tensor_scalar(out=ms[:], in0=th[:], scalar1=1e-4, scalar2=None,
                        op0=OP.is_lt)
        u1 = self.sk()
        V.scalar_tensor_tensor(out=u1[:], in0=coefb[:], scalar=-1.0, in1=ms[:],
                               op0=OP.mult, op1=OP.mult)
        u2 = self.sk()
        V.scalar_tensor_tensor(out=u2[:], in0=ms[:], scalar=1.0 / 12.0,
                               in1=coefb[:], op0=OP.mult, op1=OP.add)
        coef = self.sk()
        V.tensor_tensor(out=coef[:], in0=u1[:], in1=u2[:], op=OP.add)
        coef3 = self.s3()
        self.bcast3(coef3, coef[:])
        phi6 = self.d6()
        A.copy(b(phi6, 0, 3), out_phi3)
        self.dup(phi6)
        Et = self.eng("setail")
        px6 = self.d6()
        self.cross(phi6, te6, b(px6, 0, 3), E=Et)
        self.dup(px6)
        cpp = self.s3()
        self.cross(phi6, px6, cpp[:], E=Et)
        gg = self.s3()
        V.scalar_tensor_tensor(out=gg[:], in0=b(px6, 0, 3), scalar=-0.5,
                               in1=b(te6, 0, 3), op0=OP.mult, op1=OP.add)
        hh = self.s3()
        Et.tensor_tensor(out=hh[:], in0=coef3[:], in1=cpp[:], op=OP.mult)
        V.tensor_tensor(out=out_tau3, in0=gg[:], in1=hh[:], op=OP.add)


def build_nc(cfg):
    K, NT, NS = cfg["K"], cfg["NT"], cfg["NS"]
    nc = bass.Bass()
    din = {}
    for nm, sh, dt in [
        ("enq1", [NT, P, 4 * K], F32), ("enq2", [NT, P, 4 * K], F32),
        ("epq", [NT, P, 4 * K], F32),
        ("ent1", [NT, P, 3 * K], F16), ("ent2", [NT, P, 3 * K], F16),
        ("ept", [NT, P, 3 * K], F16),
        ("inq0", [NS, P, 4 * K], F32), ("inq1", [NS, P, 4 * K], F32),
        ("idrq", [NS, P, 4 * K], F32),
        ("int0", [NS, P, 3 * K], F16), ("int1", [NS, P, 3 * K], F16),
        ("iv0", [NS, P, 3 * K], F16), ("iv1", [NS, P, 3 * K], F16),
        ("idtr", [NS, P, 3 * K], F16), ("idv", [NS, P, 3 * K], F16),
        ("idts", [NS, P, K], F16),
    ]:
        din[nm] = nc.dram_tensor(nm, sh, dt, kind="ExternalInput")
    pg = nc.dram_tensor("pg", [NT, P, 6 * K], F16, kind="ExternalOutput")
    adj = nc.dram_tensor("adj", [NS, P, 3 * K], F16, kind="ExternalOutput")
    rot = nc.dram_tensor("rot", [NS, P, 3 * K], F16, kind="ExternalOutput")
    tvl = nc.dram_tensor("tvl", [NS, P, 3 * K], F16, kind="ExternalOutput")

    with TileContext(nc) as tc:
        with (
            tc.tile_pool(name="io_e", bufs=1) as io_e,
            tc.tile_pool(name="io_o", bufs=2) as io_o,
            tc.tile_pool(name="io_i", bufs=1) as io_i,
            tc.tile_pool(name="d6", bufs=5) as d6p,
            tc.tile_pool(name="s3", bufs=8) as s3p,
            tc.tile_pool(name="s3f", bufs=5) as s3fp,
            tc.tile_pool(name="s1", bufs=4) as s1p,
            tc.tile_pool(name="sk", bufs=12) as skp,
        ):
            pools = dict(d6=d6p, s3=s3p, s3f=s3fp, s1=s1p, sk=skp)
            em = _Emit(nc, pools, K, offload=cfg.get("offload"))
            V, A, S, b = em.V, em.A, em.S, em.blk

            def edge_tile(t):
                q1 = io_e.tile([P, 4 * K], F32, name=f"q1_{t}", tag="q1")
                q2 = io_e.tile([P, 4 * K], F32, name=f"q2_{t}", tag="q2")
                qp = io_e.tile([P, 4 * K], F32, name=f"qp_{t}", tag="qp")
                t1 = io_e.tile([P, 3 * K], F16, name=f"t1_{t}", tag="t1")
                t2 = io_e.tile([P, 3 * K], F16, name=f"t2_{t}", tag="t2")
                tp = io_e.tile([P, 3 * K], F16, name=f"tp_{t}", tag="tp")
                ot = io_o.tile([P, 6 * K], F16, name=f"ot_{t}", tag="ot")
                for tile_, nm in [(q1, "enq1"), (q2, "enq2"), (qp, "epq"),
                                  (t1, "ent1"), (t2, "ent2"), (tp, "ept")]:
                    S.dma_start(out=tile_[:], in_=din[nm][t, :, :])
                q1v, q1w = q1[:, 0:3 * K], q1[:, 3 * K:4 * K]
                q2v, q2w = q2[:, 0:3 * K], q2[:, 3 * K:4 * K]
                qpv, qpw = qp[:, 0:3 * K], qp[:, 3 * K:4 * K]
                # qa = conj(q1) (x) q2 ; qe = conj(qp) (x) qa   (fp32)
                qav = em.s3f()
                qaw = em.sk()
                em.qmul_conj_c(q1v, q1w, q2v, q2w, qav[:], qaw[:])
                qev = em.s3f()
                qew = em.sk()
                em.qmul_conj_c(qpv, qpw, qav[:], qaw[:], qev[:], qew[:])
                # fp16 copies of q1,qp for the translation rotations
                v16 = em.d6()
                A.copy(b(v16, 0, 3), q1v)
                em.dup(v16)
                w13 = em.s3()
                em.bcast3(w13, q1w)
                vp6 = em.d6()
                A.copy(b(vp6, 0, 3), qpv)
                em.dup(vp6)
                wp3 = em.s3()
                em.bcast3(wp3, qpw)
                # u = t2 - t1 ; ta = R(conj(q1)) u
                u6 = em.d6()
                V.tensor_tensor(out=b(u6, 0, 3), in0=t2[:], in1=t1[:],
                                op=OP.subtract)
                em.dup(u6)
                ta3 = em.s3()
                em.qrot_conj(v16, w13, u6, ta3[:])
                # v' = ta - tp ; te = R(conj(qp)) v'
                vv6 = em.d6()
                V.tensor_tensor(out=b(vv6, 0, 3), in0=ta3[:], in1=tp[:],
                                op=OP.subtract)
                em.dup(vv6)
                te6 = em.d6()
                em.qrot_conj(vp6, wp3, vv6, b(te6, 0, 3))
                em.dup(te6)
                em.so3_se3(qev[:], qew[:], b(ot[:], 3, 3), te6, b(ot[:], 0, 3))
                S.dma_start(out=pg[t, :, :], in_=ot[:])

            def imu_tile(s):
                tq0 = io_i.tile([P, 4 * K], F32, name=f"tq0_{s}", tag="tq0")
                tq1 = io_i.tile([P, 4 * K], F32, name=f"tq1_{s}", tag="tq1")
                tdr = io_i.tile([P, 4 * K], F32, name=f"tdr_{s}", tag="tdr")
                tt0 = io_i.tile([P, 3 * K], F16, name=f"tt0_{s}", tag="tt0")
                tt1 = io_i.tile([P, 3 * K], F16, name=f"tt1_{s}", tag="tt1")
                tv0 = io_i.tile([P, 3 * K], F16, name=f"tv0_{s}", tag="tv0")
                tv1 = io_i.tile([P, 3 * K], F16, name=f"tv1_{s}", tag="tv1")
                tdtr = io_i.tile([P, 3 * K], F16, name=f"tdtr_{s}", tag="tdtr")
                tdv = io_i.tile([P, 3 * K], F16, name=f"tdv_{s}", tag="tdv")
                tdts = io_i.tile([P, K], F16, name=f"tdts_{s}", tag="tdts")
                to_a = io_i.tile([P, 3 * K], F16, name=f"to_a_{s}", tag="to_a")
                to_r = io_i.tile([P, 3 * K], F16, name=f"to_r_{s}", tag="to_r")
                to_t = io_i.tile([P, 3 * K], F16, name=f"to_t_{s}", tag="to_t")
                for tile_, nm in [(tq0, "inq0"), (tq1, "inq1"), (tdr, "idrq"),
                                  (tt0, "int0"), (tt1, "int1"), (tv0, "iv0"),
                                  (tv1, "iv1"), (tdtr, "idtr"), (tdv, "idv"),
                                  (tdts, "idts")]:
                    S.dma_start(out=tile_[:], in_=din[nm][s, :, :])
                # part 2: adj = 0.1*(dv - (v1 - v0))
                Ei = em.eng("imu24")
                dvv = em.s3()
                Ei.tensor_tensor(out=dvv[:], in0=tv1[:], in1=tv0[:], op=OP.subtract)
                ee = em.s3()
                Ei.tensor_tensor(out=ee[:], in0=tdv[:], in1=dvv[:], op=OP.subtract)
                A.mul(out=to_a[:], in_=ee[:], mul=0.1)
                S.dma_start(out=adj[s, :, :], in_=to_a[:])
                # part 4: tvl = 0.1*((t1 - t0) - (v0*dts + dtr))
                dts3 = em.s3()
                em.bcast3(dts3, tdts[:])
                y1 = em.s3()
                Ei.tensor_tensor(out=y1[:], in0=tv0[:], in1=dts3[:], op=OP.mult)
                Ei.tensor_tensor(out=y1[:], in0=y1[:], in1=tdtr[:], op=OP.add)
                y3 = em.s3()
                Ei.tensor_tensor(out=y3[:], in0=tt1[:], in1=tt0[:], op=OP.subtract)
                Ei.tensor_tensor(out=y3[:], in0=y3[:], in1=y1[:], op=OP.subtract)
                A.mul(out=to_t[:], in_=y3[:], mul=0.1)
                S.dma_start(out=tvl[s, :, :], in_=to_t[:])
                # part 3: qre = conj(dr) (x) (conj(q0) (x) q1), fp32
                qqv = em.s3f()
                qqw = em.sk()
                em.qmul_conj_c(tq0[:, 0:3 * K], tq0[:, 3 * K:4 * K],
                               tq1[:, 0:3 * K], tq1[:, 3 * K:4 * K],
                               qqv[:], qqw[:])
                qrv = em.s3f()
                qrw = em.sk()
                em.qmul_conj_c(tdr[:, 0:3 * K], tdr[:, 3 * K:4 * K],
                               qqv[:], qqw[:], qrv[:], qrw[:])
                em.so3_se3(qrv[:], qrw[:], to_r[:])
                S.dma_start(out=rot[s, :, :], in_=to_r[:])

            for _rep in range(cfg.get("reps", 1)):
                for t in range(NT):
                    edge_tile(t)
                for s in range(NS):
                    imu_tile(s)

    nc.finalize()
    _split_excess_waits(nc)
    return nc


# ---------------- host side ----------------

def _soa(x, ns, w, dt):
    """[ns*128*w, D] -> [ns, 128, D*w] component-block layout."""
    d = x.shape[1]
    return np.ascontiguousarray(
        x.reshape(ns, P, w, d).transpose(0, 1, 3, 2).reshape(ns, P, d * w)
    ).astype(dt)


def _unsoa(y, d, rows):
    ns, _, dw = y.shape
    w = dw // d
    out = y.astype(np.float32).reshape(ns, P, d, w).transpose(0, 1, 3, 2)
    return out.reshape(ns * P * w, d)[:rows]


def kernel(edges, nodes, vels, poses, imu_drots, imu_dtrans, imu_dvels, dts,
           cfg=None, _run=None):
    cfg = cfg or FULL
    E, N, M = cfg["E"], cfg["N"], cfg["M"]
    K, NT, NS, NC = cfg["K"], cfg["NT"], cfg["NS"], cfg["NC"]
    edges = np.asarray(edges)
    nodes = np.asarray(nodes, dtype=np.float32)
    vels = np.asarray(vels, dtype=np.float32)
    poses = np.asarray(poses, dtype=np.float32)
    imu_drots = np.asarray(imu_drots, dtype=np.float32)
    imu_dtrans = np.asarray(imu_dtrans, dtype=np.float32)
    imu_dvels = np.asarray(imu_dvels, dtype=np.float32)
    dts = np.asarray(dts, dtype=np.float32)

    Epc = E // NC
    Epad = P * K * NT
    MW = P * K * NS
    identq = np.array([0, 0, 0, 1], np.float32)

    # attach endpoint node features to each edge shard
    n1 = nodes[edges[:, 0]]
    n2 = nodes[edges[:, 1]]

    nodes_pad = np.vstack([
        nodes, np.tile(np.r_[np.zeros(3, np.float32), identq],
                       (NC * MW + 1 - N, 1))])
    vels_pad = np.vstack([vels, np.zeros((NC * MW + 1 - N, 3), np.float32)])
    mpad = NC * MW - M
    dr_pad = np.vstack([imu_drots, np.tile(identq, (mpad, 1))])
    dtr_pad = np.vstack([imu_dtrans, np.zeros((mpad, 3), np.float32)])
    dv_pad = np.vstack([imu_dvels, np.zeros((mpad, 3), np.float32)])
    dts_pad = np.vstack([dts, np.ones((mpad, 1), np.float32)])

    in_maps = []
    for c in range(NC):
        sl = slice(c * Epc, (c + 1) * Epc)

        def eq(x):  # [Epc,7] -> quaternion part tiles, f32
            xp = np.tile(identq, (Epad, 1))
            xp[:Epc] = x[sl][:, 3:7]
            return _soa(xp, NT, K, np.float32)

        def et(x):  # translation part tiles, f16
            xp = np.zeros((Epad, 3), np.float32)
            xp[:Epc] = x[sl][:, 0:3]
            return _soa(xp, NT, K, np.float16)

        base = c * MW
        m = dict(
            enq1=eq(n1), enq2=eq(n2), epq=eq(poses),
            ent1=et(n1), ent2=et(n2), ept=et(poses),
            inq0=_soa(nodes_pad[base:base + MW, 3:7], NS, K, np.float32),
            inq1=_soa(nodes_pad[base + 1:base + MW + 1, 3:7], NS, K, np.float32),
            idrq=_soa(dr_pad[base:base + MW], NS, K, np.float32),
            int0=_soa(nodes_pad[base:base + MW, 0:3], NS, K, np.float16),
            int1=_soa(nodes_pad[base + 1:base + MW + 1, 0:3], NS, K, np.float16),
            iv0=_soa(vels_pad[base:base + MW], NS, K, np.float16),
            iv1=_soa(vels_pad[base + 1:base + MW + 1], NS, K, np.float16),
            idtr=_soa(dtr_pad[base:base + MW], NS, K, np.float16),
            idv=_soa(dv_pad[base:base + MW], NS, K, np.float16),
            idts=_soa(dts_pad[base:base + MW], NS, K, np.float16),
        )
        in_maps.append(m)

    if _run is None:
        from concourse.bass_utils import run_bass_kernel_spmd
        nc = build_nc(cfg)
        res = run_bass_kernel_spmd(nc, in_maps, core_ids=list(range(NC)))
        global LAST_RESULT
        LAST_RESULT = res
        outs = res.results
    else:
        outs = _run(in_maps)

    pgs, adjs, rots, tvls = [], [], [], []
    for c in range(NC):
        o = outs[c]
        pgs.append(_unsoa(o["pg"].reshape(NT, P, 6 * K), 6, Epc))
        adjs.append(_unsoa(o["adj"], 3, MW))
        rots.append(_unsoa(o["rot"], 3, MW))
        tvls.append(_unsoa(o["tvl"], 3, MW))
    pg_full = np.concatenate(pgs, axis=0)
    adj_full = np.concatenate(adjs, axis=0)[:M]
    rot_full = np.concatenate(rots, axis=0)[:M]
    tvl_full = np.concatenate(tvls, axis=0)[:M]
    return np.concatenate([
        pg_full.ravel(), adj_full.ravel(), rot_full.ravel(), tvl_full.ravel()
    ]).astype(np.float32)
